# revision 5
# baseline (speedup 1.0000x reference)
"""MoE layer (24 experts, top-2 routing) on 8 Trainium2 NeuronCores.

Expert-parallel sharding: the host computes the gate routing (scores -> top-2
-> softmax combine weights), then dispatches each expert's tokens to the core
that owns the expert (3 experts per core, count-balanced by a sort-descending
assignment).  Each core runs one SPMD Bass/Tile program that, for each of its
3 expert slots, computes

    H^T[f, t] = gelu(w1^T-contract(x^T) + b1)      (MM1, K = d_model = 1024)
    Y^T[d, t] = w2^T-contract(H^T) + b2            (MM2, K = d_ff    = 4096)
    out       = Y^T * gate_weight[t]

with tokens on the matmul FREE dim, so per-expert token counts need no
128-padding (capacity = max count per slot across cores, rounded to even).
The host scatter-adds the per-expert outputs back into the [T, d] output
(the "combine" side of the all-to-all).

Matmuls run in bf16 (weights, x, and h), accumulating in fp32 PSUM: the PE
streams 1 row/cycle either way, but bf16 halves the dominant HBM traffic -
the expert weights (100 MB/core fp32 -> 50 MB/core bf16), turning the kernel
from DMA-bound into PE-bound.  Matmul rel-error ~2e-3, far inside the 2e-2
gate.  Weight DMAs are batched into 2 MiB transfers (8 MM1 f-tiles / 2 MM2
d-tiles per DMA) for near-peak HBM efficiency, alternating between the two
HWDGE rings (SP and ACT issuing engines).  Biases, gate weights, PSUM, and
the output stay fp32.

Host-side work is routing/dispatch/combine only (index math, gather,
scatter-add); all FLOPs of the MoE layer itself (both matmuls, gelu, biases,
gate weighting) run on device.
"""

import sys

for _p in ("/opt/trn_rl_repo", "/root/.axon_site/_ro/trn_rl_repo"):
    if _p not in sys.path:
        sys.path.append(_p)

import ml_dtypes
import numpy as np

import concourse.tile as tile
from concourse import bacc, mybir
from concourse.bass_utils import run_bass_kernel_spmd

B, S, D, FF, E, TOPK = 4, 1024, 1024, 4096, 24, 2
T = B * S
P = 128
KT1 = D // P     # 8  k-subtiles for MM1
MT1 = FF // P    # 32 f-tiles (MM1 output partition tiles)
KT2 = FF // P    # 32 k-subtiles for MM2
MT2 = D // P     # 8  d-tiles (MM2 output partition tiles)
W1G = 8          # MM1 f-tiles per weight DMA (2 MiB bf16 per transfer)
G1 = MT1 // W1G  # 4 w1 DMA groups
W2G = 2          # MM2 d-tiles per weight DMA (2 MiB bf16 per transfer)
G2 = MT2 // W2G  # 4 w2 DMA groups
N_CORES = 8
SLOTS = E // N_CORES  # 3 experts per core

BF16 = mybir.dt.bfloat16
F32 = mybir.dt.float32
NP_BF16 = ml_dtypes.bfloat16

_program_cache: dict = {}


def _build_program(caps, loop_reps=None, bench_internal=False):
    """One SPMD program: SLOTS expert slots with token capacities caps[j].

    loop_reps: replicate the body N times (benchmark-only, to measure the
    steady-state device time via a wall-clock slope over N).
    bench_internal: benchmark-only - every tensor lives in internal DRAM
    scratch (plus one tiny ExternalOutput so the program has I/O), so
    wall-clock timing excludes host<->device shipping while keeping
    an identical per-rep instruction stream and DMA traffic.
    """
    nc = bacc.Bacc("TRN2", target_bir_lowering=False, debug=False)

    kin = "Internal" if bench_internal else "ExternalInput"
    kout = "Internal" if bench_internal else "ExternalOutput"
    sfx = "_int" if bench_internal else ""
    w1t = nc.dram_tensor("w1t" + sfx, (SLOTS, G1, P, W1G, KT1, P), BF16, kind=kin)
    w2t = nc.dram_tensor("w2t" + sfx, (SLOTS, G2, P, W2G, KT2, P), BF16, kind=kin)
    b1t = nc.dram_tensor("b1t" + sfx, (SLOTS, P, MT1), F32, kind=kin)
    b2t = nc.dram_tensor("b2t" + sfx, (SLOTS, P, MT2), F32, kind=kin)
    xgs = [nc.dram_tensor(f"xg{j}" + sfx, (P, KT1, caps[j]), BF16, kind=kin)
           for j in range(SLOTS)]
    gws = [nc.dram_tensor(f"gw{j}" + sfx, (P, caps[j]), F32, kind=kin)
           for j in range(SLOTS)]
    ygs = [nc.dram_tensor(f"yg{j}" + sfx, (MT2, P, caps[j]), F32, kind=kout)
           for j in range(SLOTS)]
    tick = (nc.dram_tensor("tick", (1, 2), F32, kind="ExternalOutput")
            if bench_internal else None)

    with tile.TileContext(nc) as tc:
        with tc.tile_pool(name="xg", bufs=SLOTS + 1) as xg_pool, \
             tc.tile_pool(name="gw", bufs=SLOTS + 1) as gw_pool, \
             tc.tile_pool(name="bias", bufs=SLOTS + 1) as bias_pool, \
             tc.tile_pool(name="w1", bufs=3) as w1_pool, \
             tc.tile_pool(name="w2", bufs=3) as w2_pool, \
             tc.tile_pool(name="h", bufs=MT1) as h_pool, \
             tc.tile_pool(name="epi", bufs=4) as epi_pool, \
             tc.tile_pool(name="psa", bufs=4, space="PSUM") as psa, \
             tc.tile_pool(name="psb", bufs=4, space="PSUM") as psb:
            for _rep in range(loop_reps or 1):
                dma_rr = [0]

                def wdma(dst, src):
                    # alternate weight DMAs across the two HWDGE rings
                    eng = nc.scalar if (dma_rr[0] % 2) else nc.sync
                    dma_rr[0] += 1
                    eng.dma_start(dst, src)

                # Preload every slot's activations/gates/biases so slot
                # boundaries never wait on small DMAs.
                slot_in = []
                for j in range(SLOTS):
                    C = caps[j]
                    xg_sb = xg_pool.tile([P, KT1, C], BF16, tag="xg")
                    nc.sync.dma_start(xg_sb[:], xgs[j].ap()[:])
                    gw_sb = gw_pool.tile([P, C], F32, tag="gw")
                    nc.sync.dma_start(gw_sb[:], gws[j].ap()[:])
                    b1_sb = bias_pool.tile([P, MT1], F32, tag="b1")
                    nc.sync.dma_start(b1_sb[:], b1t.ap()[j])
                    b2_sb = bias_pool.tile([P, MT2], F32, tag="b2")
                    nc.sync.dma_start(b2_sb[:], b2t.ap()[j])
                    slot_in.append((xg_sb, gw_sb, b1_sb, b2_sb))

                for j in range(SLOTS):
                    C = caps[j]
                    xg_sb, gw_sb, b1_sb, b2_sb = slot_in[j]

                    # Phase A: H^T tiles, one 128-row f-tile at a time.
                    h_tiles = []
                    for g in range(G1):
                        w1_sb = w1_pool.tile([P, W1G, KT1, P], BF16, tag="w1")
                        wdma(w1_sb[:], w1t.ap()[j, g])
                        for mi in range(W1G):
                            m = g * W1G + mi
                            ph = psa.tile([P, C], F32, tag="psa")
                            for k in range(KT1):
                                nc.tensor.matmul(ph[:], w1_sb[:, mi, k, :],
                                                 xg_sb[:, k, :],
                                                 start=(k == 0),
                                                 stop=(k == KT1 - 1))
                            h_sb = h_pool.tile([P, C], BF16, tag="h")
                            nc.scalar.activation(h_sb[:], ph[:],
                                                 mybir.ActivationFunctionType.Gelu,
                                                 bias=b1_sb[:, m:m + 1])
                            h_tiles.append(h_sb)

                    # Phase B: Y^T tiles; epilogue adds b2, scales by gate.
                    for go in range(G2):
                        w2_sb = w2_pool.tile([P, W2G, KT2, P], BF16, tag="w2")
                        wdma(w2_sb[:], w2t.ap()[j, go])
                        for mi in range(W2G):
                            mo = go * W2G + mi
                            py = psb.tile([P, C], F32, tag="psb")
                            for k in range(KT2):
                                nc.tensor.matmul(py[:], w2_sb[:, mi, k, :],
                                                 h_tiles[k][:],
                                                 start=(k == 0),
                                                 stop=(k == KT2 - 1))
                            # fused epilogue on DVE: yo = (py + b2) * gw
                            # (single PSUM reader in phase B; ACT keeps gelu)
                            yo = epi_pool.tile([P, C], F32, tag="yo")
                            nc.vector.scalar_tensor_tensor(
                                yo[:], py[:], b2_sb[:, mo:mo + 1], gw_sb[:],
                                op0=mybir.AluOpType.add,
                                op1=mybir.AluOpType.mult)
                            # store via SWDGE (gpsimd) so the compute-gated
                            # store's sem-wait never blocks the HWDGE load
                            # rings at rep/expert boundaries
                            nc.gpsimd.dma_start(ygs[j].ap()[mo], yo[:])
            if tick is not None:
                # tiny I/O so the bench program has an ExternalOutput; reads
                # an already-written tile (Tile forbids reading unwritten SBUF)
                nc.sync.dma_start(tick.ap()[:], slot_in[-1][1][0:1, 0:2])
    nc.compile()
    return nc


def _route(x2d, gate_w, gate_b):
    """fp32 gate scores -> top-2 indices -> softmax combine weights."""
    scores = x2d @ gate_w + gate_b                               # [T, E]
    topi = np.argsort(-scores, axis=1, kind="stable")[:, :TOPK]  # [T, 2]
    topv = np.take_along_axis(scores, topi, axis=1)
    g = np.exp(topv - topv.max(axis=1, keepdims=True))
    g = g / g.sum(axis=1, keepdims=True)
    return topi, g.astype(np.float32)


def kernel(x, gate_w, gate_b, w1, b1, w2, b2):
    x = np.ascontiguousarray(np.asarray(x, dtype=np.float32))
    gate_w = np.asarray(gate_w, dtype=np.float32)
    gate_b = np.asarray(gate_b, dtype=np.float32)
    w1 = np.asarray(w1, dtype=np.float32)
    b1 = np.asarray(b1, dtype=np.float32)
    w2 = np.asarray(w2, dtype=np.float32)
    b2 = np.asarray(b2, dtype=np.float32)

    x2d = x.reshape(T, D)
    topi, gates = _route(x2d, gate_w, gate_b)

    # Token list and combine weight per expert (token order preserved).
    idx_e = [np.nonzero(topi == e)[0] for e in range(E)]
    gv_e = []
    for e in range(E):
        rows = topi == e                       # [T, 2] bool, <=1 True per row
        sel = rows.any(axis=1)
        gv_e.append(gates[sel, :][rows[sel, :]].astype(np.float32))
    counts = np.array([len(i) for i in idx_e])

    # Balance experts over (core, slot): sort by count descending; slot j
    # holds ranks [8j, 8j+8).  Slot capacity = max count in the slot,
    # rounded up to even.
    order = np.argsort(-counts, kind="stable")
    slot_expert = np.empty((N_CORES, SLOTS), dtype=int)
    caps = []
    for j in range(SLOTS):
        ranks = order[j * N_CORES:(j + 1) * N_CORES]
        slot_expert[:, j] = ranks
        cmax = int(counts[ranks].max())
        caps.append(cmax + (cmax & 1))
    caps = tuple(caps)

    if caps not in _program_cache:
        _program_cache[caps] = _build_program(caps)
    nc = _program_cache[caps]

    xTb = np.ascontiguousarray(x2d.T).astype(NP_BF16)      # [D, T] bf16
    in_maps = []
    for c in range(N_CORES):
        m = {}
        w1c = np.empty((SLOTS, G1, P, W1G, KT1, P), NP_BF16)
        w2c = np.empty((SLOTS, G2, P, W2G, KT2, P), NP_BF16)
        b1c = np.empty((SLOTS, P, MT1), np.float32)
        b2c = np.empty((SLOTS, P, MT2), np.float32)
        for j in range(SLOTS):
            e = int(slot_expert[c, j])
            C = caps[j]
            n = int(counts[e])
            xg = np.zeros((P, KT1, C), NP_BF16)
            xg[:, :, :n] = xTb[:, idx_e[e]].reshape(KT1, P, n).transpose(1, 0, 2)
            m[f"xg{j}"] = xg
            gw = np.zeros((C,), np.float32)
            gw[:n] = gv_e[e]
            m[f"gw{j}"] = np.broadcast_to(gw, (P, C)).copy()
            # weight tiles in the exact SBUF layouts for single clean DMAs
            t1 = w1[e].reshape(KT1, P, MT1, P).transpose(2, 1, 0, 3)
            w1c[j] = t1.reshape(G1, W1G, P, KT1, P).transpose(0, 2, 1, 3, 4)
            t2 = w2[e].reshape(KT2, P, MT2, P).transpose(2, 1, 0, 3)
            w2c[j] = t2.reshape(G2, W2G, P, KT2, P).transpose(0, 2, 1, 3, 4)
            b1c[j] = b1[e].reshape(MT1, P).T
            b2c[j] = b2[e].reshape(MT2, P).T
        m["w1t"] = w1c
        m["w2t"] = w2c
        m["b1t"] = b1c
        m["b2t"] = b2c
        in_maps.append(m)

    res = run_bass_kernel_spmd(nc, in_maps, core_ids=list(range(N_CORES)))

    # Combine: scatter-add each expert's weighted outputs back to tokens.
    out = np.zeros((T, D), np.float32)
    for c in range(N_CORES):
        for j in range(SLOTS):
            e = int(slot_expert[c, j])
            n = int(counts[e])
            yg = res.results[c][f"yg{j}"].reshape(D, caps[j])
            out[idx_e[e], :] += yg[:, :n].T
    return out.reshape(B, S, D)
